# revision 15
# baseline (speedup 1.0000x reference)
"""Trainium2 Bass kernel for dense_cnn problem.

Math (per batch element n, C=128 channels, H=W=56, G=8):
  t1 = conv_h(x, w1)          5-tap conv over H with full channel mixing
  t3 = dwconv_h(t1, w3)       3-tap depthwise conv over H
  t4[g] = sum_{c,k} x[c, h, w+2k-2] * w4[c,k,g]   (3 width taps, dil 2)
  out[c] = t3[c] * t4[c % 8]

Device strategy (data-parallel, 4 batch elems per core across 8 cores):
  - PE does the dense work: t1 as a 5-tap conv (clipped shifted matmuls)
    and t4 broadcast to 128 channels (3 taps) -> 8 column passes per
    chunk.  That is the engine floor (~42us @ 2.4GHz); the back-end is
    arranged so every other engine stays well under the PE's
    1.5us/chunk.
  - PSUM tiles hold a PAIR of 8-row chunks as [C, 2, 512] (one bank per
    chunk, 448 used + 64 pad) so the ACT engine can drain both matmul
    outputs with ONE copy per pair (fixed overhead amortized):
      t1s = w3[c,1] * t1(pair)     ACT, PSUM->SBUF bf16 halo tile
      t4s = t4(pair)               ACT, PSUM->SBUF bf16
  - Depthwise 3-tap + final multiply on DVE ops that have fast uop
    modes (measured: scalar_tensor_tensor is ALWAYS 1x; tensor_scalar
    with a per-partition ptr reaches 4x; tensor_tensor reaches 2x on
    bf16; PSUM operands force 1x, so everything reads SBUF):
      a   = (w30/w31) * t1s[h-1]      tensor_scalar ptr, 4x
      q   = a + t1s[h]                tensor_tensor,     2x
      b   = (w32/w31) * t1s[h+1]      tensor_scalar ptr, 4x
      t3  = b + q                     tensor_tensor,     2x
      out = t3 * t4s       (fp16)     tensor_tensor,     2x
    t1s has zero pad rows, so no border special cases.  All five ops
    are batched over the 2-chunk granule.  Per-chunk budget: PE 1500ns,
    DVE ~1090ns, ACT ~890ns, GPS 0.
  - b/t3/mul for granule g need t1s rows from the next granule's first
    ACT copy, so that half of the pipeline runs one granule behind.
  - Head: engine sequencers are busy with program load until ~7us and
    the first input DMA's completion semaphore cannot fire before
    ~10us (descriptor streaming + HBM completion latency), so 8 warm-up
    matmuls on a memset tile keep the PE busy 7.7->10.7us.  This trips
    the HAM clock gate (1.2 -> 2.4 GHz after ~3.4us of *sustained* PE
    activity - any gap restarts the window), so the real matmuls run
    at full clock from the start.
  - Matmuls in bf16 (fp32 matmul lowers to a LOW_HIGH pair at <half
    throughput); accumulation stays fp32 in PSUM.
  - Output written fp16 (half the DMA bytes), widened on host.
  - x for batch elems 1..3 is DMA'd as one transfer each (6272B
    per-partition descriptors run near peak BW); batch 0 is sliced
    finer so chunk 0 can start as early as possible.
"""

import sys

sys.path.insert(0, "/opt/trn_rl_repo")

import ml_dtypes
import numpy as np

import concourse.bacc as bacc
import concourse.bass as bass
import concourse.mybir as mybir
import concourse.tile as tile
from concourse import bass_utils

N, C, H, W, G = 32, 128, 56, 56, 8
NCORES = 8
NPC = N // NCORES  # batch elems per core
CH = 8             # H rows per chunk
NCHUNK = H // CH
BANK = 512         # fp32 elems per PSUM bank (2KB)

F32 = mybir.dt.float32
F16 = mybir.dt.float16
BF16 = mybir.dt.bfloat16

TRACE = False
TRACE_DIR = None
LAST_EXEC_NS = None
LAST_RESULTS = None

_COMPILED = None


def _enable_trace_hook():
    """The agent image's ``antenv`` lacks ``axon_hooks``, so the boot-time
    NTFF hook registration silently degraded. Recreate the module and
    register the same ctypes-based hook; also skip the bucket upload."""
    import sys as _sys
    import types

    if "antenv.axon_hooks" not in _sys.modules:
        mod = types.ModuleType("antenv.axon_hooks")
        mod._hook = None

        def set_axon_ntff_profile_hook(h):
            mod._hook = h

        def get_axon_ntff_profile_hook():
            return mod._hook

        mod.set_axon_ntff_profile_hook = set_axon_ntff_profile_hook
        mod.get_axon_ntff_profile_hook = get_axon_ntff_profile_hook
        _sys.modules["antenv.axon_hooks"] = mod
        import antenv

        antenv.axon_hooks = mod

    from antenv.axon_hooks import get_axon_ntff_profile_hook as _get

    if _get() is None:
        from trn_agent_boot.trn_boot import _ntff_profile_via_ctypes

        hook = _ntff_profile_via_ctypes("/opt/axon/libaxon_pjrt.so")
        if hook is not None:
            _sys.modules["antenv.axon_hooks"].set_axon_ntff_profile_hook(hook)

    bass_utils.upload_artifacts = lambda tmpdir: f"local:{tmpdir}"


def _chunk_view(ps, j):
    """[C, CH, W] view of chunk half j of a [C, 2, BANK] pair tile."""
    return ps[:, j, 0 : CH * W].rearrange("p (h w) -> p h w", w=W)


def _t1_matmuls(c, paj, xc, wc_t):
    """5-tap H-conv for chunk c with row clipping at the H borders.
    Output row o of the chunk reads x row 8c+o+e-2 for tap e."""
    h0 = c * CH
    mms = []
    # e=2 covers the full chunk for every c -> emitted first (start=True)
    for e in (2, 0, 1, 3, 4):
        o_lo = max(0, 2 - e - h0)
        o_hi = min(CH, H + 2 - e - h0)
        if o_lo >= o_hi:
            continue
        r0 = h0 + o_lo + e - 2
        r1 = h0 + o_hi + e - 2
        mms.append((wc_t[:, e, :], xc[:, r0:r1, :], paj[:, o_lo:o_hi, :]))
    return mms


def _t4_matmuls(c, pbj, xc, w4_t):
    """t4 chunk: 3 width taps at offsets -2/0/+2, col-clipped at borders."""
    h0 = c * CH
    rows = xc[:, h0 : h0 + CH, :]
    return [
        (w4_t[:, 1, :], rows, pbj[:]),                               # delta = 0
        (w4_t[:, 0, :], xc[:, h0 : h0 + CH, 0 : W - 2], pbj[:, :, 2:W]),   # -2
        (w4_t[:, 2, :], xc[:, h0 : h0 + CH, 2:W], pbj[:, :, 0 : W - 2]),   # +2
    ]


def _build():
    nc = bacc.Bacc(
        "TRN2",
        target_bir_lowering=False,
        debug=False,
        enable_asserts=False,
        num_devices=NCORES,
    )

    x_d = nc.dram_tensor("x_s", (NPC, C, H, W), BF16, kind="ExternalInput").ap()
    wc_d = nc.dram_tensor("wc5", (C, 5, C), BF16, kind="ExternalInput").ap()
    w4_d = nc.dram_tensor("w4b", (C, 3, C), BF16, kind="ExternalInput").ap()
    sc_d = nc.dram_tensor("scal", (C, 3), F32, kind="ExternalInput").ap()
    out_d = nc.dram_tensor("out", (NPC, C, H, W), F16, kind="ExternalOutput").ap()

    COPY = mybir.ActivationFunctionType.Copy

    # 2-chunk back-end granules (last one is a 1-chunk remainder)
    GRAN = [(0, 2), (2, 2), (4, 2), (6, 1)]

    with tile.TileContext(nc) as tc:
        with (
            tc.tile_pool(name="wpool", bufs=1) as wpool,
            tc.tile_pool(name="xpool", bufs=1) as xpool,
            tc.tile_pool(name="t1pool", bufs=2) as t1pool,
            tc.tile_pool(name="t4sb", bufs=2) as t4pool,
            tc.tile_pool(name="apool", bufs=2) as apool,
            tc.tile_pool(name="qpool", bufs=2) as qpool,
            tc.tile_pool(name="t3pool", bufs=2) as t3pool,
            tc.tile_pool(name="opool", bufs=3) as opool,
            tc.tile_pool(name="psA", bufs=2, space="PSUM") as papool,
            tc.tile_pool(name="psB", bufs=2, space="PSUM") as pbpool,
        ):
            # Warm-up matmuls: keep the PE busy from sequencer-ready
            # (~7.7us) until the first input data semaphore (~10.5us) so
            # the HAM busy-window fills and real work runs at 2.4 GHz.
            # Results land in a PSUM region that is never read.
            dmy = wpool.tile([C, BANK], BF16)
            nc.gpsimd.memset(dmy[:], 0.0)
            dps = papool.tile([C, 2, BANK], F32, name="pa2")
            for _ in range(8):
                nc.tensor.matmul(
                    dps[:, 0, :], lhsT=dmy[:, 0:C], rhs=dmy[:],
                    start=True, stop=True,
                )

            wc_t = wpool.tile([C, 5, C], BF16)
            w4_t = wpool.tile([C, 3, C], BF16)
            sc_t = wpool.tile([C, 3], F32)
            warm = wpool.tile([1, 1], F32)

            xcs = []
            for n in range(NPC):
                xc = xpool.tile([C, H, W], BF16, name=f"xc{n}")
                xcs.append(xc)

            # DMA order: weights, then x in few large transfers (6272B
            # per-partition descriptors run near peak BW).  The warm-up
            # matmuls cover the time until x batch 0's first semaphore
            # fires, so fine-grained slicing (which risks pre-HAM PE gaps
            # that reset the clock-gate busy window) is not needed.
            nc.sync.dma_start(wc_t[:], wc_d[:])
            nc.sync.dma_start(sc_t[:], sc_d[:])
            nc.sync.dma_start(xcs[0][:, 0:16, :], x_d[0, :, 0:16, :])
            nc.sync.dma_start(w4_t[:], w4_d[:])
            nc.sync.dma_start(xcs[0][:, 16:56, :], x_d[0, :, 16:56, :])
            for n in range(1, NPC):
                nc.sync.dma_start(xcs[n][:], x_d[n])

            # Trip the one-time ACT_TABLE_LOAD (~1.3us) before the first
            # real copy needs it.
            nc.scalar.activation(warm[:], sc_t[0:1, 0:1], COPY)

            w31 = sc_t[:, 0:1]
            s0 = sc_t[:, 1:2]
            s2 = sc_t[:, 2:3]

            for n in range(NPC):
                xc = xcs[n]
                last_n = n == NPC - 1
                # Last batch elem: per-chunk back-end granules so the
                # post-last-matmul drain chain covers only 8 rows.
                gran = [(c, 1) for c in range(NCHUNK)] if last_n else GRAN

                # t1s rows: 0 = zero pad (h=-1), 1..56 = h, 57 = zero pad
                t1s = t1pool.tile([C, H + 2, W], BF16, name="t1s")
                nc.gpsimd.memset(t1s[:, 0:1, :], 0.0)
                nc.gpsimd.memset(t1s[:, H + 1 : H + 2, :], 0.0)

                pas = {}
                pbs = {}
                t4ss = {}
                qs = {}

                def front(c):
                    """PE matmuls for chunk c into half c%2 of the pair
                    PSUM tiles."""
                    g, j = divmod(c, 2)
                    if j == 0:
                        pas[g] = papool.tile([C, 2, BANK], F32, name="pa2")
                        pbs[g] = pbpool.tile([C, 2, BANK], F32, name="pb2")
                    paj = _chunk_view(pas[g], j)
                    mms = _t1_matmuls(c, paj, xc, wc_t)
                    for i, (lhsT, rhs, outap) in enumerate(mms):
                        nc.tensor.matmul(
                            outap, lhsT=lhsT, rhs=rhs,
                            start=(i == 0), stop=(i == len(mms) - 1),
                        )
                    pbj = _chunk_view(pbs[g], j)
                    for i, (lhsT, rhs, outap) in enumerate(_t4_matmuls(c, pbj, xc, w4_t)):
                        nc.tensor.matmul(
                            outap, lhsT=lhsT, rhs=rhs,
                            start=(i == 0), stop=(i == 2),
                        )

                def copies(gi, skip_b=False):
                    """ACT drains the (half-)pair PSUM tiles: t1s halo
                    (scaled by w31) and t4s (plain cast to bf16)."""
                    c0, ln = gran[gi]
                    h0 = c0 * CH
                    g, j = divmod(c0, 2)
                    src_a = pas[g][:, j : j + ln, 0 : CH * W].rearrange(
                        "p a (h w) -> p a h w", w=W
                    )
                    dst_a = t1s[:, 1 + h0 : 1 + h0 + ln * CH, :].rearrange(
                        "p (a h) w -> p a h w", a=ln
                    )
                    nc.scalar.activation(dst_a, src_a, COPY, scale=w31)
                    if skip_b:
                        return
                    src_b = pbs[g][:, j : j + ln, 0 : CH * W].rearrange(
                        "p a (h w) -> p a h w", w=W
                    )
                    t4s = t4pool.tile([C, 2 * CH, W], BF16, name="t4s")
                    dst_b = t4s[:, 0 : ln * CH, :].rearrange(
                        "p (a h) w -> p a h w", a=ln
                    )
                    nc.scalar.activation(dst_b, src_b, COPY)
                    t4ss[gi] = t4s

                def aq(gi):
                    """a = s0*t1s[h-1] (DVE 4x), q = a + t1s[h] (DVE 2x)."""
                    c0, ln = gran[gi]
                    h0 = c0 * CH
                    rows = ln * CH
                    a = apool.tile([C, 2 * CH, W], BF16, name="a")
                    nc.vector.tensor_scalar_mul(
                        a[:, 0:rows, :], t1s[:, h0 : h0 + rows, :], s0
                    )
                    q = qpool.tile([C, 2 * CH, W], BF16, name="q")
                    nc.vector.tensor_add(
                        q[:, 0:rows, :],
                        a[:, 0:rows, :],
                        t1s[:, 1 + h0 : 1 + h0 + rows, :],
                    )
                    qs[gi] = q

                def back(gi, psum_mul=False):
                    """b = s2*t1s[h+1] (4x), t3 = b + q (2x), out = t3*t4s
                    (2x, fp16), store.  Needs the next granule's first ACT
                    copy (zero pad row for the last granule).  psum_mul
                    multiplies straight against the PSUM tile instead of
                    t4s (slower on DVE, but drops the ACT copy from the
                    end-of-kernel critical chain)."""
                    c0, ln = gran[gi]
                    h0 = c0 * CH
                    rows = ln * CH
                    b = t3pool.tile([C, 2 * CH, W], BF16, name="b")
                    nc.vector.tensor_scalar_mul(
                        b[:, 0:rows, :], t1s[:, 2 + h0 : 2 + h0 + rows, :], s2
                    )
                    t3 = t3pool.tile([C, 2 * CH, W], BF16, name="t3")
                    nc.vector.tensor_add(
                        t3[:, 0:rows, :], b[:, 0:rows, :], qs[gi][:, 0:rows, :]
                    )
                    ot = opool.tile([C, 2 * CH, W], F16, name="ot")
                    if psum_mul:
                        g, j = divmod(c0, 2)
                        t4src = _chunk_view(pbs[g], j)
                    else:
                        t4src = t4ss[gi][:, 0:rows, :]
                    nc.vector.tensor_mul(
                        ot[:, 0:rows, :], t3[:, 0:rows, :], t4src
                    )
                    nc.sync.dma_start(
                        out_d[n, :, h0 : h0 + rows, :], ot[:, 0:rows, :]
                    )

                if not last_n:
                    front(0)
                    front(1)
                    copies(0)
                    aq(0)
                    front(2)
                    front(3)
                    copies(1)
                    aq(1)
                    back(0)
                    front(4)
                    front(5)
                    copies(2)
                    aq(2)
                    back(1)
                    front(6)
                    copies(3)
                    aq(3)
                    back(2)
                    back(3)
                else:
                    # Per-chunk pipeline: back(c) needs copies(c+1)'s t1s
                    # row, so it runs one chunk behind.  The very last
                    # chunk multiplies straight from PSUM so the final
                    # chain has only one ACT copy in it.
                    for c in range(NCHUNK):
                        front(c)
                        copies(c, skip_b=(c == NCHUNK - 1))
                        aq(c)
                        if c >= 1:
                            back(c - 1)
                    back(NCHUNK - 1, psum_mul=True)

    nc.compile()
    return nc


def _get_compiled():
    global _COMPILED
    if _COMPILED is None:
        _COMPILED = _build()
    return _COMPILED


def _prep_weights(w1, w3, w4):
    bf = ml_dtypes.bfloat16
    w1c = np.asarray(w1, dtype=np.float32)[:, :, :, 0]  # (co, ci, 5)
    wc5 = np.ascontiguousarray(np.transpose(w1c, (1, 2, 0))).astype(bf)  # (ci,e,co)
    w4c = np.asarray(w4, dtype=np.float32)[:, :, 0, :]  # (ci, k, g)
    w4b = np.ascontiguousarray(np.tile(w4c, (1, 1, C // G))).astype(bf)
    w3c = np.asarray(w3, dtype=np.float32)[:, 0, :, 0]  # (co, 3)
    w31 = w3c[:, 1].copy()
    w31[np.abs(w31) < 1e-12] = 1e-12
    scal = np.stack([w31, w3c[:, 0] / w31, w3c[:, 2] / w31], axis=1)
    return wc5, w4b, np.ascontiguousarray(scal, dtype=np.float32)


def kernel(x, w1, w3, w4):
    global LAST_EXEC_NS, LAST_RESULTS
    nc = _get_compiled()
    xb = np.ascontiguousarray(np.asarray(x, dtype=np.float32)).astype(ml_dtypes.bfloat16)
    wc5, w4b, scal = _prep_weights(w1, w3, w4)

    in_maps = [
        {
            "x_s": np.ascontiguousarray(xb[i * NPC : (i + 1) * NPC]),
            "wc5": wc5,
            "w4b": w4b,
            "scal": scal,
        }
        for i in range(NCORES)
    ]
    if TRACE:
        _enable_trace_hook()
    res = bass_utils.run_bass_kernel_spmd(
        nc,
        in_maps,
        core_ids=list(range(NCORES)),
        trace=TRACE,
        tmpdir=TRACE_DIR,
    )
    LAST_EXEC_NS = res.exec_time_ns
    LAST_RESULTS = res
    out = np.concatenate(
        [res.results[i]["out"].astype(np.float32) for i in range(NCORES)], axis=0
    )
    return out


# revision 17
# speedup vs baseline: 1.0485x; 1.0485x over previous
"""Trainium2 Bass kernel for dense_cnn problem.

Math (per batch element n, C=128 channels, H=W=56, G=8):
  t1 = conv_h(x, w1)          5-tap conv over H with full channel mixing
  t3 = dwconv_h(t1, w3)       3-tap depthwise conv over H
  t4[g] = sum_{c,k} x[c, h, w+2k-2] * w4[c,k,g]   (3 width taps, dil 2)
  out[c] = t3[c] * t4[c % 8]

Device strategy (data-parallel, 4 batch elems per core across 8 cores):
  - PE does the dense work: t1 as a 5-tap conv (clipped shifted matmuls)
    and t4 broadcast to 128 channels (3 taps) -> 8 column passes per
    chunk.  That is the engine floor (~42us @ 2.4GHz); the back-end is
    arranged so every other engine stays well under the PE's
    1.5us/chunk.
  - PSUM tiles hold a PAIR of 8-row chunks as [C, 2, 512] (one bank per
    chunk, 448 used + 64 pad) so the ACT engine can drain both matmul
    outputs with ONE copy per pair (fixed overhead amortized):
      t1s = w3[c,1] * t1(pair)     ACT, PSUM->SBUF bf16 halo tile
      t4s = t4(pair)               ACT, PSUM->SBUF bf16
  - Depthwise 3-tap + final multiply on DVE ops that have fast uop
    modes (measured: scalar_tensor_tensor is ALWAYS 1x; tensor_scalar
    with a per-partition ptr reaches 4x; tensor_tensor reaches 2x on
    bf16; PSUM operands force 1x, so everything reads SBUF):
      a   = (w30/w31) * t1s[h-1]      tensor_scalar ptr, 4x
      q   = a + t1s[h]                tensor_tensor,     2x
      b   = (w32/w31) * t1s[h+1]      tensor_scalar ptr, 4x
      t3  = b + q                     tensor_tensor,     2x
      out = t3 * t4s       (fp16)     tensor_tensor,     2x
    t1s has zero pad rows, so no border special cases.  All five ops
    are batched over the 2-chunk granule.  Per-chunk budget: PE 1500ns,
    DVE ~1090ns, ACT ~890ns, GPS 0.
  - b/t3/mul for granule g need t1s rows from the next granule's first
    ACT copy, so that half of the pipeline runs one granule behind.
  - Head: engine sequencers are busy with program load until ~7us and
    the first input DMA's completion semaphore cannot fire before
    ~10us (descriptor streaming + HBM completion latency), so 8 warm-up
    matmuls on a memset tile keep the PE busy 7.7->10.7us.  This trips
    the HAM clock gate (1.2 -> 2.4 GHz after ~3.4us of *sustained* PE
    activity - any gap restarts the window), so the real matmuls run
    at full clock from the start.
  - Matmuls in bf16 (fp32 matmul lowers to a LOW_HIGH pair at <half
    throughput); accumulation stays fp32 in PSUM.
  - Output written fp16 (half the DMA bytes), widened on host.
  - x for batch elems 1..3 is DMA'd as one transfer each (6272B
    per-partition descriptors run near peak BW); batch 0 is sliced
    finer so chunk 0 can start as early as possible.
"""

import sys

sys.path.insert(0, "/opt/trn_rl_repo")

import ml_dtypes
import numpy as np

import concourse.bacc as bacc
import concourse.bass as bass
import concourse.mybir as mybir
import concourse.tile as tile
from concourse import bass_utils

N, C, H, W, G = 32, 128, 56, 56, 8
NCORES = 8
NPC = N // NCORES  # batch elems per core
CH = 8             # H rows per chunk
NCHUNK = H // CH
BANK = 512         # fp32 elems per PSUM bank (2KB)

F32 = mybir.dt.float32
F16 = mybir.dt.float16
BF16 = mybir.dt.bfloat16

TRACE = False
TRACE_DIR = None
LAST_EXEC_NS = None
LAST_RESULTS = None

_COMPILED = None


def _enable_trace_hook():
    """The agent image's ``antenv`` lacks ``axon_hooks``, so the boot-time
    NTFF hook registration silently degraded. Recreate the module and
    register the same ctypes-based hook; also skip the bucket upload."""
    import sys as _sys
    import types

    if "antenv.axon_hooks" not in _sys.modules:
        mod = types.ModuleType("antenv.axon_hooks")
        mod._hook = None

        def set_axon_ntff_profile_hook(h):
            mod._hook = h

        def get_axon_ntff_profile_hook():
            return mod._hook

        mod.set_axon_ntff_profile_hook = set_axon_ntff_profile_hook
        mod.get_axon_ntff_profile_hook = get_axon_ntff_profile_hook
        _sys.modules["antenv.axon_hooks"] = mod
        import antenv

        antenv.axon_hooks = mod

    from antenv.axon_hooks import get_axon_ntff_profile_hook as _get

    if _get() is None:
        from trn_agent_boot.trn_boot import _ntff_profile_via_ctypes

        hook = _ntff_profile_via_ctypes("/opt/axon/libaxon_pjrt.so")
        if hook is not None:
            _sys.modules["antenv.axon_hooks"].set_axon_ntff_profile_hook(hook)

    bass_utils.upload_artifacts = lambda tmpdir: f"local:{tmpdir}"


def _chunk_view(ps, j):
    """[C, CH, W] view of chunk half j of a [C, 2, BANK] pair tile."""
    return ps[:, j, 0 : CH * W].rearrange("p (h w) -> p h w", w=W)


def _t1_matmuls(c, paj, xc, wc_t):
    """5-tap H-conv for chunk c with row clipping at the H borders.
    Output row o of the chunk reads x row 8c+o+e-2 for tap e."""
    h0 = c * CH
    mms = []
    # e=2 covers the full chunk for every c -> emitted first (start=True)
    for e in (2, 0, 1, 3, 4):
        o_lo = max(0, 2 - e - h0)
        o_hi = min(CH, H + 2 - e - h0)
        if o_lo >= o_hi:
            continue
        r0 = h0 + o_lo + e - 2
        r1 = h0 + o_hi + e - 2
        mms.append((wc_t[:, e, :], xc[:, r0:r1, :], paj[:, o_lo:o_hi, :]))
    return mms


def _t4_matmuls(c, pbj, xc, w4_t):
    """t4 chunk: 3 width taps at offsets -2/0/+2, col-clipped at borders."""
    h0 = c * CH
    rows = xc[:, h0 : h0 + CH, :]
    return [
        (w4_t[:, 1, :], rows, pbj[:]),                               # delta = 0
        (w4_t[:, 0, :], xc[:, h0 : h0 + CH, 0 : W - 2], pbj[:, :, 2:W]),   # -2
        (w4_t[:, 2, :], xc[:, h0 : h0 + CH, 2:W], pbj[:, :, 0 : W - 2]),   # +2
    ]


def _build():
    nc = bacc.Bacc(
        "TRN2",
        target_bir_lowering=False,
        debug=False,
        enable_asserts=False,
        num_devices=NCORES,
    )

    x_d = nc.dram_tensor("x_s", (NPC, C, H, W), BF16, kind="ExternalInput").ap()
    wc_d = nc.dram_tensor("wc5", (C, 5, C), BF16, kind="ExternalInput").ap()
    w4_d = nc.dram_tensor("w4b", (C, 3, C), BF16, kind="ExternalInput").ap()
    sc_d = nc.dram_tensor("scal", (C, 3), F32, kind="ExternalInput").ap()
    out_d = nc.dram_tensor("out", (NPC, C, H, W), F16, kind="ExternalOutput").ap()

    COPY = mybir.ActivationFunctionType.Copy

    # 2-chunk back-end granules (last one is a 1-chunk remainder)
    GRAN = [(0, 2), (2, 2), (4, 2), (6, 1)]

    with tile.TileContext(nc) as tc:
        with (
            tc.tile_pool(name="wpool", bufs=1) as wpool,
            tc.tile_pool(name="xpool", bufs=1) as xpool,
            tc.tile_pool(name="t1pool", bufs=2) as t1pool,
            tc.tile_pool(name="t4sb", bufs=2) as t4pool,
            tc.tile_pool(name="apool", bufs=2) as apool,
            tc.tile_pool(name="qpool", bufs=2) as qpool,
            tc.tile_pool(name="t3pool", bufs=2) as t3pool,
            tc.tile_pool(name="opool", bufs=3) as opool,
            tc.tile_pool(name="psA", bufs=2, space="PSUM") as papool,
            tc.tile_pool(name="psB", bufs=2, space="PSUM") as pbpool,
        ):
            # Warm-up matmuls: keep the PE busy from sequencer-ready
            # (~7.7us) until the first input data semaphore (~10.5us) so
            # the HAM busy-window fills and real work runs at 2.4 GHz.
            # Results land in a PSUM region that is never read.
            dmy = wpool.tile([C, BANK], BF16)
            nc.gpsimd.memset(dmy[:], 0.0)
            dps = papool.tile([C, 2, BANK], F32, name="pa2")
            for _ in range(9):
                nc.tensor.matmul(
                    dps[:, 0, :], lhsT=dmy[:, 0:C], rhs=dmy[:],
                    start=True, stop=True,
                )

            wc_t = wpool.tile([C, 5, C], BF16)
            w4_t = wpool.tile([C, 3, C], BF16)
            sc_t = wpool.tile([C, 3], F32)
            warm = wpool.tile([1, 1], F32)

            xcs = []
            for n in range(NPC):
                xc = xpool.tile([C, H, W], BF16, name=f"xc{n}")
                xcs.append(xc)

            # DMA order: weights, then x in few large transfers (6272B
            # per-partition descriptors run near peak BW).  The warm-up
            # matmuls cover the time until x batch 0's first semaphore
            # fires, so fine-grained slicing (which risks pre-HAM PE gaps
            # that reset the clock-gate busy window) is not needed.
            nc.sync.dma_start(wc_t[:, 2, :], wc_d[:, 2, :])
            nc.sync.dma_start(xcs[0][:, 0:8, :], x_d[0, :, 0:8, :])
            nc.sync.dma_start(wc_t[:, 0:2, :], wc_d[:, 0:2, :])
            nc.sync.dma_start(xcs[0][:, 8:10, :], x_d[0, :, 8:10, :])
            nc.sync.dma_start(wc_t[:, 3:5, :], wc_d[:, 3:5, :])
            nc.sync.dma_start(w4_t[:], w4_d[:])
            nc.sync.dma_start(sc_t[:], sc_d[:])
            nc.sync.dma_start(xcs[0][:, 10:28, :], x_d[0, :, 10:28, :])
            nc.sync.dma_start(xcs[0][:, 28:56, :], x_d[0, :, 28:56, :])
            for n in range(1, NPC):
                nc.sync.dma_start(xcs[n][:], x_d[n])

            # Trip the one-time ACT_TABLE_LOAD (~1.3us) before the first
            # real copy needs it.
            nc.scalar.activation(warm[:], sc_t[0:1, 0:1], COPY)

            w31 = sc_t[:, 0:1]
            s0 = sc_t[:, 1:2]
            s2 = sc_t[:, 2:3]

            for n in range(NPC):
                xc = xcs[n]
                last_n = n == NPC - 1
                # Last batch elem: per-chunk back-end granules so the
                # post-last-matmul drain chain covers only 8 rows.
                gran = [(c, 1) for c in range(NCHUNK)] if last_n else GRAN

                # t1s rows: 0 = zero pad (h=-1), 1..56 = h, 57 = zero pad
                t1s = t1pool.tile([C, H + 2, W], BF16, name="t1s")
                nc.gpsimd.memset(t1s[:, 0:1, :], 0.0)
                nc.gpsimd.memset(t1s[:, H + 1 : H + 2, :], 0.0)

                pas = {}
                pbs = {}
                t4ss = {}
                qs = {}

                def front(c):
                    """PE matmuls for chunk c into half c%2 of the pair
                    PSUM tiles."""
                    g, j = divmod(c, 2)
                    if j == 0:
                        pas[g] = papool.tile([C, 2, BANK], F32, name="pa2")
                        pbs[g] = pbpool.tile([C, 2, BANK], F32, name="pb2")
                    paj = _chunk_view(pas[g], j)
                    mms = _t1_matmuls(c, paj, xc, wc_t)
                    for i, (lhsT, rhs, outap) in enumerate(mms):
                        nc.tensor.matmul(
                            outap, lhsT=lhsT, rhs=rhs,
                            start=(i == 0), stop=(i == len(mms) - 1),
                        )
                    pbj = _chunk_view(pbs[g], j)
                    for i, (lhsT, rhs, outap) in enumerate(_t4_matmuls(c, pbj, xc, w4_t)):
                        nc.tensor.matmul(
                            outap, lhsT=lhsT, rhs=rhs,
                            start=(i == 0), stop=(i == 2),
                        )

                def copies(gi, skip_b=False):
                    """ACT drains the (half-)pair PSUM tiles: t1s halo
                    (scaled by w31) and t4s (plain cast to bf16)."""
                    c0, ln = gran[gi]
                    h0 = c0 * CH
                    g, j = divmod(c0, 2)
                    src_a = pas[g][:, j : j + ln, 0 : CH * W].rearrange(
                        "p a (h w) -> p a h w", w=W
                    )
                    dst_a = t1s[:, 1 + h0 : 1 + h0 + ln * CH, :].rearrange(
                        "p (a h) w -> p a h w", a=ln
                    )
                    nc.scalar.activation(dst_a, src_a, COPY, scale=w31)
                    if skip_b:
                        return
                    src_b = pbs[g][:, j : j + ln, 0 : CH * W].rearrange(
                        "p a (h w) -> p a h w", w=W
                    )
                    t4s = t4pool.tile([C, 2 * CH, W], BF16, name="t4s")
                    dst_b = t4s[:, 0 : ln * CH, :].rearrange(
                        "p (a h) w -> p a h w", a=ln
                    )
                    nc.scalar.activation(dst_b, src_b, COPY)
                    t4ss[gi] = t4s

                def aq(gi):
                    """a = s0*t1s[h-1] (DVE 4x), q = a + t1s[h] (DVE 2x)."""
                    c0, ln = gran[gi]
                    h0 = c0 * CH
                    rows = ln * CH
                    a = apool.tile([C, 2 * CH, W], BF16, name="a")
                    nc.vector.tensor_scalar_mul(
                        a[:, 0:rows, :], t1s[:, h0 : h0 + rows, :], s0
                    )
                    q = qpool.tile([C, 2 * CH, W], BF16, name="q")
                    nc.vector.tensor_add(
                        q[:, 0:rows, :],
                        a[:, 0:rows, :],
                        t1s[:, 1 + h0 : 1 + h0 + rows, :],
                    )
                    qs[gi] = q

                def back(gi, psum_mul=False):
                    """b = s2*t1s[h+1] (4x), t3 = b + q (2x), out = t3*t4s
                    (2x, fp16), store.  Needs the next granule's first ACT
                    copy (zero pad row for the last granule).  psum_mul
                    multiplies straight against the PSUM tile instead of
                    t4s (slower on DVE, but drops the ACT copy from the
                    end-of-kernel critical chain)."""
                    c0, ln = gran[gi]
                    h0 = c0 * CH
                    rows = ln * CH
                    b = t3pool.tile([C, 2 * CH, W], BF16, name="b")
                    nc.vector.tensor_scalar_mul(
                        b[:, 0:rows, :], t1s[:, 2 + h0 : 2 + h0 + rows, :], s2
                    )
                    t3 = t3pool.tile([C, 2 * CH, W], BF16, name="t3")
                    nc.vector.tensor_add(
                        t3[:, 0:rows, :], b[:, 0:rows, :], qs[gi][:, 0:rows, :]
                    )
                    ot = opool.tile([C, 2 * CH, W], F16, name="ot")
                    if psum_mul:
                        g, j = divmod(c0, 2)
                        t4src = _chunk_view(pbs[g], j)
                    else:
                        t4src = t4ss[gi][:, 0:rows, :]
                    nc.vector.tensor_mul(
                        ot[:, 0:rows, :], t3[:, 0:rows, :], t4src
                    )
                    nc.sync.dma_start(
                        out_d[n, :, h0 : h0 + rows, :], ot[:, 0:rows, :]
                    )

                if not last_n:
                    front(0)
                    front(1)
                    copies(0)
                    aq(0)
                    front(2)
                    front(3)
                    copies(1)
                    aq(1)
                    back(0)
                    front(4)
                    front(5)
                    copies(2)
                    aq(2)
                    back(1)
                    front(6)
                    copies(3)
                    aq(3)
                    back(2)
                    back(3)
                else:
                    # Per-chunk pipeline: back(c) needs copies(c+1)'s t1s
                    # row, so it runs one chunk behind.  The very last
                    # chunk multiplies straight from PSUM so the final
                    # chain has only one ACT copy in it.
                    for c in range(NCHUNK):
                        front(c)
                        copies(c, skip_b=(c == NCHUNK - 1))
                        aq(c)
                        if c >= 1:
                            back(c - 1)
                    back(NCHUNK - 1, psum_mul=True)

    nc.compile()
    return nc


def _get_compiled():
    global _COMPILED
    if _COMPILED is None:
        _COMPILED = _build()
    return _COMPILED


def _prep_weights(w1, w3, w4):
    bf = ml_dtypes.bfloat16
    w1c = np.asarray(w1, dtype=np.float32)[:, :, :, 0]  # (co, ci, 5)
    wc5 = np.ascontiguousarray(np.transpose(w1c, (1, 2, 0))).astype(bf)  # (ci,e,co)
    w4c = np.asarray(w4, dtype=np.float32)[:, :, 0, :]  # (ci, k, g)
    w4b = np.ascontiguousarray(np.tile(w4c, (1, 1, C // G))).astype(bf)
    w3c = np.asarray(w3, dtype=np.float32)[:, 0, :, 0]  # (co, 3)
    w31 = w3c[:, 1].copy()
    w31[np.abs(w31) < 1e-12] = 1e-12
    scal = np.stack([w31, w3c[:, 0] / w31, w3c[:, 2] / w31], axis=1)
    return wc5, w4b, np.ascontiguousarray(scal, dtype=np.float32)


def kernel(x, w1, w3, w4):
    global LAST_EXEC_NS, LAST_RESULTS
    nc = _get_compiled()
    xb = np.ascontiguousarray(np.asarray(x, dtype=np.float32)).astype(ml_dtypes.bfloat16)
    wc5, w4b, scal = _prep_weights(w1, w3, w4)

    in_maps = [
        {
            "x_s": np.ascontiguousarray(xb[i * NPC : (i + 1) * NPC]),
            "wc5": wc5,
            "w4b": w4b,
            "scal": scal,
        }
        for i in range(NCORES)
    ]
    if TRACE:
        _enable_trace_hook()
    res = bass_utils.run_bass_kernel_spmd(
        nc,
        in_maps,
        core_ids=list(range(NCORES)),
        trace=TRACE,
        tmpdir=TRACE_DIR,
    )
    LAST_EXEC_NS = res.exec_time_ns
    LAST_RESULTS = res
    out = np.concatenate(
        [res.results[i]["out"].astype(np.float32) for i in range(NCORES)], axis=0
    )
    return out
